# revision 1
# baseline (speedup 1.0000x reference)
"""HeteroGNN (2-layer GCN x 4 stacks) on 8 Trainium2 NeuronCores.

Sharding: cores {2s, 2s+1} handle stack s (jac-lnc, jac-prot, blast-lnc,
blast-prot); within a pair, destination nodes are split in halves of 25000.
Each GCN layer runs transform-first: xw = x @ W computed on-device, rows
pre-scaled by dinv[n]; per-edge aggregation is a gather (indirect DMA) of
xw rows + one-hot matmul scatter into PSUM per 128-destination group.
The inter-layer halo exchange (pair halves of xw2) goes through the host
between two SPMD launches; the final view-combine is elementwise on host.
"""
import os
import sys
import time

sys.path.insert(0, "/opt/trn_rl_repo")

import numpy as np

import concourse.bass as bass
import concourse.mybir as mybir
import concourse.tile as tile
from concourse import bacc
from concourse.bass_utils import run_bass_kernel_spmd
from concourse.masks import make_identity

N = 50000
NP = 50176          # padded (392 * 128)
HALF = 25000
HP = 25088          # padded half (196 * 128)
NG = 196            # dst groups per half
NCH = 392           # node chunks for the dense transform
F_IN = 256
HID = 256
OUT = 128
P = 128

F32 = mybir.dt.float32
I32 = mybir.dt.int32

LAST_EXEC_NS = []   # filled when KERNEL_TRACE=1


def _install_ntff_hook():
    """Register the axon NTFF profile hook (the image's antenv lacks it) and
    neuter the S3 artifact upload so tracing works offline."""
    import types, contextlib, ctypes
    import antenv  # noqa: F401
    mod = types.ModuleType("antenv.axon_hooks")
    holder = {"hook": None}
    mod.set_axon_ntff_profile_hook = lambda h: holder.__setitem__("hook", h)
    mod.get_axon_ntff_profile_hook = lambda: holder["hook"]
    sys.modules["antenv.axon_hooks"] = mod
    lib = ctypes.CDLL("/opt/axon/libaxon_pjrt.so")
    lib.axon_start_nrt_profile.argtypes = [ctypes.POINTER(ctypes.c_int64), ctypes.c_size_t]
    lib.axon_start_nrt_profile.restype = ctypes.c_int64
    lib.axon_stop_nrt_profile.argtypes = [ctypes.c_char_p]
    lib.axon_stop_nrt_profile.restype = ctypes.c_int64

    @contextlib.contextmanager
    def _hook(output_dir, device_ids):
        import jax
        jax.devices()
        if device_ids:
            ids = (ctypes.c_int64 * len(device_ids))(*device_ids)
            rc = lib.axon_start_nrt_profile(ids, len(device_ids))
        else:
            rc = lib.axon_start_nrt_profile(None, 0)
        if rc != 0:
            raise RuntimeError(f"axon_start_nrt_profile rc={rc}")
        try:
            yield
        finally:
            lib.axon_stop_nrt_profile(str(output_dir).encode())

    mod.set_axon_ntff_profile_hook(_hook)
    from concourse import bass_utils
    bass_utils.upload_artifacts = lambda tmpdir: str(tmpdir)


def _build_a(Kg, cs, nblk):
    nc = bacc.Bacc("TRN2", target_bir_lowering=False, debug=False, num_devices=8)
    xT = nc.dram_tensor("xT", [F_IN, NP], F32, kind="ExternalInput")
    W1 = nc.dram_tensor("W1", [F_IN, HID], F32, kind="ExternalInput")
    W2 = nc.dram_tensor("W2", [HID, OUT], F32, kind="ExternalInput")
    Wr = nc.dram_tensor("Wr", [F_IN, OUT], F32, kind="ExternalInput")
    b1t_d = nc.dram_tensor("b1t", [P, HID], F32, kind="ExternalInput")
    iota_d = nc.dram_tensor("iota", [P, P], F32, kind="ExternalInput")
    idx_d = nc.dram_tensor("idx", [P, nblk], I32, kind="ExternalInput")
    dmod_d = nc.dram_tensor("dmod", [P, nblk], F32, kind="ExternalInput")
    dnod_d = nc.dram_tensor("dnod", [P, NCH], F32, kind="ExternalInput")
    ddst_d = nc.dram_tensor("ddst", [P, NG], F32, kind="ExternalInput")
    xw2_o = nc.dram_tensor("xw2s_own", [HP, OUT], F32, kind="ExternalOutput")
    res_o = nc.dram_tensor("res_own", [HP, OUT], F32, kind="ExternalOutput")
    xw1s = nc.dram_tensor("xw1s", [NP, HID], F32)

    with tile.TileContext(nc) as tc:
        with (
            tc.tile_pool(name="const", bufs=1) as cp,
            tc.tile_pool(name="xt", bufs=6) as xtp,
            tc.tile_pool(name="mm1", bufs=2, space="PSUM") as mm1p,
            tc.tile_pool(name="sb1", bufs=4) as sb1p,
            tc.tile_pool(name="data", bufs=12) as datap,
            tc.tile_pool(name="oh", bufs=12) as ohp,
            tc.tile_pool(name="agg", bufs=2, space="PSUM") as aggp,
            tc.tile_pool(name="tp", bufs=2, space="PSUM") as tpp,
            tc.tile_pool(name="mm2", bufs=2, space="PSUM") as mm2p,
            tc.tile_pool(name="ep", bufs=3) as epp,
        ):
            w1a = cp.tile([P, HID], F32); nc.sync.dma_start(out=w1a[:], in_=W1[0:P, :])
            w1b = cp.tile([P, HID], F32); nc.sync.dma_start(out=w1b[:], in_=W1[P:2 * P, :])
            w2a = cp.tile([P, OUT], F32); nc.sync.dma_start(out=w2a[:], in_=W2[0:P, :])
            w2b = cp.tile([P, OUT], F32); nc.sync.dma_start(out=w2b[:], in_=W2[P:2 * P, :])
            wra = cp.tile([P, OUT], F32); nc.sync.dma_start(out=wra[:], in_=Wr[0:P, :])
            wrb = cp.tile([P, OUT], F32); nc.sync.dma_start(out=wrb[:], in_=Wr[P:2 * P, :])
            b1t = cp.tile([P, HID], F32); nc.sync.dma_start(out=b1t[:], in_=b1t_d[:])
            iota = cp.tile([P, P], F32); nc.sync.dma_start(out=iota[:], in_=iota_d[:])
            idx_t = cp.tile([P, nblk], I32); nc.sync.dma_start(out=idx_t[:], in_=idx_d[:])
            dmod_t = cp.tile([P, nblk], F32); nc.sync.dma_start(out=dmod_t[:], in_=dmod_d[:])
            dn_t = cp.tile([P, NCH], F32); nc.sync.dma_start(out=dn_t[:], in_=dnod_d[:])
            dd_t = cp.tile([P, NG], F32); nc.sync.dma_start(out=dd_t[:], in_=ddst_d[:])
            ident = cp.tile([P, P], F32); make_identity(nc, ident[:])

            # step 1: xw1s[n] = dinv[n] * (x[n] @ W1) for all nodes
            for cc in range(NCH // 4):
                xa = xtp.tile([P, 4 * P], F32, tag="xt")
                nc.sync.dma_start(out=xa[:], in_=xT[0:P, cc * 4 * P:(cc + 1) * 4 * P])
                xb = xtp.tile([P, 4 * P], F32, tag="xt")
                nc.sync.dma_start(out=xb[:], in_=xT[P:2 * P, cc * 4 * P:(cc + 1) * 4 * P])
                for j in range(4):
                    c = cc * 4 + j
                    ps = mm1p.tile([P, HID], F32)
                    nc.tensor.matmul(out=ps[:], lhsT=xa[:, j * P:(j + 1) * P], rhs=w1a[:],
                                     start=True, stop=False)
                    nc.tensor.matmul(out=ps[:], lhsT=xb[:, j * P:(j + 1) * P], rhs=w1b[:],
                                     start=False, stop=True)
                    t = sb1p.tile([P, HID], F32)
                    nc.vector.tensor_tensor(out=t[:], in0=dn_t[:, c:c + 1].to_broadcast([P, HID]),
                                            in1=ps[:], op=mybir.AluOpType.mult)
                    nc.sync.dma_start(out=xw1s[c * P:(c + 1) * P, :], in_=t[:])

            tc.strict_bb_all_engine_barrier()

            # step 2: per dst-group scatter + epilogue + transform to xw2
            for g in range(NG):
                agg = aggp.tile([P, HID], F32)
                for b in range(Kg[g]):
                    blk = cs[g] + b
                    data = datap.tile([P, HID], F32, tag="data")
                    nc.gpsimd.indirect_dma_start(
                        out=data[:], out_offset=None, in_=xw1s[:],
                        in_offset=bass.IndirectOffsetOnAxis(ap=idx_t[:, blk:blk + 1], axis=0))
                    oh = ohp.tile([P, P], F32, tag="oh")
                    nc.vector.tensor_tensor(out=oh[:], in0=dmod_t[:, blk:blk + 1].to_broadcast([P, P]),
                                            in1=iota[:], op=mybir.AluOpType.is_equal)
                    nc.tensor.matmul(out=agg[:], lhsT=oh[:], rhs=data[:],
                                     start=(b == 0), stop=(b == Kg[g] - 1))
                slf = datap.tile([P, HID], F32, tag="data")
                nc.sync.dma_start(out=slf[:], in_=xw1s[g * P:(g + 1) * P, :])
                t = epp.tile([P, HID], F32, tag="h")
                nc.vector.tensor_add(out=t[:], in0=agg[:], in1=slf[:])
                nc.vector.tensor_tensor(out=t[:], in0=dd_t[:, g:g + 1].to_broadcast([P, HID]),
                                        in1=t[:], op=mybir.AluOpType.mult)
                nc.vector.tensor_add(out=t[:], in0=t[:], in1=b1t[:])
                t2 = epp.tile([P, HID], F32, tag="h2")
                nc.scalar.mul(out=t2[:], in_=t[:], mul=0.2)
                nc.vector.tensor_tensor(out=t[:], in0=t[:], in1=t2[:], op=mybir.AluOpType.max)
                # transpose h chunk, transform to xw2, scale by dinv
                hta = epp.tile([P, P], F32, tag="hta")
                htb = epp.tile([P, P], F32, tag="htb")
                pt = tpp.tile([P, P], F32, tag="pt")
                nc.tensor.transpose(out=pt[:], in_=t[:, 0:P], identity=ident[:])
                nc.vector.tensor_copy(out=hta[:], in_=pt[:])
                pt2 = tpp.tile([P, P], F32, tag="pt")
                nc.tensor.transpose(out=pt2[:], in_=t[:, P:2 * P], identity=ident[:])
                nc.vector.tensor_copy(out=htb[:], in_=pt2[:])
                ps2 = mm2p.tile([P, OUT], F32, tag="ps2")
                nc.tensor.matmul(out=ps2[:], lhsT=hta[:], rhs=w2a[:], start=True, stop=False)
                nc.tensor.matmul(out=ps2[:], lhsT=htb[:], rhs=w2b[:], start=False, stop=True)
                xw2t = epp.tile([P, OUT], F32, tag="xw2t")
                nc.vector.tensor_tensor(out=xw2t[:], in0=dd_t[:, g:g + 1].to_broadcast([P, OUT]),
                                        in1=ps2[:], op=mybir.AluOpType.mult)
                nc.sync.dma_start(out=xw2_o[g * P:(g + 1) * P, :], in_=xw2t[:])
                # residual for this chunk of own nodes
                ra = xtp.tile([P, P], F32, tag="res")
                nc.sync.dma_start(out=ra[:], in_=xT[0:P, g * P:(g + 1) * P])
                rb = xtp.tile([P, P], F32, tag="res")
                nc.sync.dma_start(out=rb[:], in_=xT[P:2 * P, g * P:(g + 1) * P])
                ps3 = mm2p.tile([P, OUT], F32, tag="ps2")
                nc.tensor.matmul(out=ps3[:], lhsT=ra[:], rhs=wra[:], start=True, stop=False)
                nc.tensor.matmul(out=ps3[:], lhsT=rb[:], rhs=wrb[:], start=False, stop=True)
                rt = epp.tile([P, OUT], F32, tag="rt")
                nc.vector.tensor_copy(out=rt[:], in_=ps3[:])
                nc.sync.dma_start(out=res_o[g * P:(g + 1) * P, :], in_=rt[:])
    nc.compile()
    return nc


def _build_b(Kg, cs, nblk):
    nc = bacc.Bacc("TRN2", target_bir_lowering=False, debug=False, num_devices=8)
    xw2f = nc.dram_tensor("xw2f", [NP, OUT], F32, kind="ExternalInput")
    b2t_d = nc.dram_tensor("b2t", [P, OUT], F32, kind="ExternalInput")
    iota_d = nc.dram_tensor("iota", [P, P], F32, kind="ExternalInput")
    idx_d = nc.dram_tensor("idx", [P, nblk], I32, kind="ExternalInput")
    dmod_d = nc.dram_tensor("dmod", [P, nblk], F32, kind="ExternalInput")
    ddst_d = nc.dram_tensor("ddst", [P, NG], F32, kind="ExternalInput")
    out_o = nc.dram_tensor("out_own", [HP, OUT], F32, kind="ExternalOutput")

    with tile.TileContext(nc) as tc:
        with (
            tc.tile_pool(name="const", bufs=1) as cp,
            tc.tile_pool(name="data", bufs=16) as datap,
            tc.tile_pool(name="oh", bufs=16) as ohp,
            tc.tile_pool(name="agg", bufs=4, space="PSUM") as aggp,
            tc.tile_pool(name="ep", bufs=3) as epp,
        ):
            b2t = cp.tile([P, OUT], F32); nc.sync.dma_start(out=b2t[:], in_=b2t_d[:])
            iota = cp.tile([P, P], F32); nc.sync.dma_start(out=iota[:], in_=iota_d[:])
            idx_t = cp.tile([P, nblk], I32); nc.sync.dma_start(out=idx_t[:], in_=idx_d[:])
            dmod_t = cp.tile([P, nblk], F32); nc.sync.dma_start(out=dmod_t[:], in_=dmod_d[:])
            dd_t = cp.tile([P, NG], F32); nc.sync.dma_start(out=dd_t[:], in_=ddst_d[:])

            for g in range(NG):
                agg = aggp.tile([P, OUT], F32)
                for b in range(Kg[g]):
                    blk = cs[g] + b
                    data = datap.tile([P, OUT], F32, tag="data")
                    nc.gpsimd.indirect_dma_start(
                        out=data[:], out_offset=None, in_=xw2f[:],
                        in_offset=bass.IndirectOffsetOnAxis(ap=idx_t[:, blk:blk + 1], axis=0))
                    oh = ohp.tile([P, P], F32, tag="oh")
                    nc.vector.tensor_tensor(out=oh[:], in0=dmod_t[:, blk:blk + 1].to_broadcast([P, P]),
                                            in1=iota[:], op=mybir.AluOpType.is_equal)
                    nc.tensor.matmul(out=agg[:], lhsT=oh[:], rhs=data[:],
                                     start=(b == 0), stop=(b == Kg[g] - 1))
                slf = datap.tile([P, OUT], F32, tag="data")
                nc.sync.dma_start(out=slf[:], in_=xw2f[g * P:(g + 1) * P, :])
                t = epp.tile([P, OUT], F32, tag="t")
                nc.vector.tensor_add(out=t[:], in0=agg[:], in1=slf[:])
                nc.vector.tensor_tensor(out=t[:], in0=dd_t[:, g:g + 1].to_broadcast([P, OUT]),
                                        in1=t[:], op=mybir.AluOpType.mult)
                nc.vector.tensor_add(out=t[:], in0=t[:], in1=b2t[:])
                t2 = epp.tile([P, OUT], F32, tag="t2")
                nc.scalar.mul(out=t2[:], in_=t[:], mul=0.2)
                nc.vector.tensor_tensor(out=t[:], in0=t[:], in1=t2[:], op=mybir.AluOpType.max)
                nc.sync.dma_start(out=out_o[g * P:(g + 1) * P, :], in_=t[:])
    nc.compile()
    return nc


def _edge_arrays(src, dst, half, cs, nblk):
    """Block-major gather indices + dst one-hot selectors for one core."""
    sel = (dst >= half * HALF) & (dst < (half + 1) * HALF)
    s = src[sel].astype(np.int64)
    l = (dst[sel] - half * HALF).astype(np.int64)
    g = l // P
    order = np.argsort(g, kind="stable")
    s, l, g = s[order], l[order], g[order]
    counts = np.bincount(g, minlength=NG)
    starts = np.zeros(NG, dtype=np.int64)
    starts[1:] = np.cumsum(counts)[:-1]
    j = np.arange(len(s)) - starts[g]
    blk = cs[g] + j // P
    p = j % P
    idx = np.zeros((P, nblk), dtype=np.int32)
    dmod = np.full((P, nblk), 255.0, dtype=np.float32)
    idx[p, blk] = s
    dmod[p, blk] = (l % P).astype(np.float32)
    return idx, dmod


def kernel(**inputs):
    trace = os.environ.get("KERNEL_TRACE", "0") == "1"
    if trace:
        try:
            _install_ntff_hook()
        except Exception:
            trace = False

    f32 = np.float32
    stacks = [
        ("x_lnc_jac", "edge_jac_lnc", "W_j1_lnc", "b_j1_lnc", "W_j2_lnc", "b_j2_lnc", "W_res_lnc"),
        ("x_prot_jac", "edge_jac_prot", "W_j1_prot", "b_j1_prot", "W_j2_prot", "b_j2_prot", "W_res_prot"),
        ("x_lnc_blast", "edge_blast_lnc", "W_b1_lnc", "b_b1_lnc", "W_b2_lnc", "b_b2_lnc", None),
        ("x_prot_blast", "edge_blast_prot", "W_b1_prot", "b_b1_prot", "W_b2_prot", "b_b2_prot", None),
    ]
    iota = np.broadcast_to(np.arange(P, dtype=f32), (P, P)).copy()

    # per-core edge preprocessing; the block structure must be uniform across
    # the SPMD program, so per-group block counts take the max over all 8 cores
    pre = []
    cnts_all = np.zeros((8, NG), dtype=np.int64)
    for c in range(8):
        sname = stacks[c // 2]
        e = np.asarray(inputs[sname[1]])
        src, dst = e[0].astype(np.int64), e[1].astype(np.int64)
        deg = (np.bincount(dst, minlength=N) + 1.0).astype(f32)
        dinv = (1.0 / np.sqrt(deg)).astype(f32)
        sel = (dst >= (c % 2) * HALF) & (dst < ((c % 2) + 1) * HALF)
        cnts_all[c] = np.bincount((dst[sel] - (c % 2) * HALF) // P, minlength=NG)
        pre.append((src, dst, dinv))
    Kg = np.maximum((cnts_all.max(axis=0) + P - 1) // P, 1).astype(np.int64)
    cs = np.zeros(NG, dtype=np.int64)
    cs[1:] = np.cumsum(Kg)[:-1]
    nblk = int(Kg.sum())

    in_a, in_b_partial = [], []
    for c in range(8):
        sname = stacks[c // 2]
        half = c % 2
        src, dst, dinv = pre[c]
        x = np.asarray(inputs[sname[0]], dtype=f32)
        # node order per core: own half first -> self rows are g*128..g*128+128
        xp = np.concatenate([x[half * HALF:], x[:half * HALF]], axis=0)
        dinv_p = np.concatenate([dinv[half * HALF:], dinv[:half * HALF]])
        xT = np.zeros((F_IN, NP), dtype=f32)
        xT[:, :N] = xp.T
        src_r = (src - half * HALF) % N
        idx, dmod = _edge_arrays(src_r, dst, half, cs, nblk)
        dpad = np.concatenate([dinv_p, np.ones(NP - N, dtype=f32)])
        dnod = dpad[np.arange(NP, dtype=np.int64).reshape(NCH, P).T].astype(f32)
        ddst = dpad[np.arange(HP, dtype=np.int64).reshape(NG, P).T].astype(f32)
        W1 = np.asarray(inputs[sname[2]], dtype=f32)
        b1 = np.asarray(inputs[sname[3]], dtype=f32)
        W2 = np.asarray(inputs[sname[4]], dtype=f32)
        b2 = np.asarray(inputs[sname[5]], dtype=f32)
        Wr = np.asarray(inputs[sname[6]], dtype=f32) if sname[6] else np.zeros((F_IN, OUT), dtype=f32)
        in_a.append({
            "xT": xT, "W1": W1, "W2": W2, "Wr": Wr,
            "b1t": np.broadcast_to(b1, (P, HID)).copy(), "iota": iota,
            "idx": idx, "dmod": dmod,
            "dnod": np.ascontiguousarray(dnod), "ddst": np.ascontiguousarray(ddst),
        })
        in_b_partial.append({
            "b2t": np.broadcast_to(b2, (P, OUT)).copy(), "iota": iota,
            "idx": idx, "dmod": dmod,
            "ddst": np.ascontiguousarray(ddst),
        })

    import tempfile
    nc_a = _build_a(Kg, cs, nblk)
    res_a = run_bass_kernel_spmd(nc_a, in_a, list(range(8)), trace=trace,
                                 tmpdir=tempfile.mkdtemp(prefix="gnn_a_") if trace else None)
    LAST_EXEC_NS.clear()
    if trace and res_a.exec_time_ns:
        LAST_EXEC_NS.append(res_a.exec_time_ns)

    # host halo exchange: assemble full xw2 per pair
    in_b = []
    for c in range(8):
        partner = c ^ 1
        xw2f = np.zeros((NP, OUT), dtype=f32)
        xw2f[:HALF] = res_a.results[c]["xw2s_own"][:HALF]
        xw2f[HALF:N] = res_a.results[partner]["xw2s_own"][:HALF]
        in_b.append({"xw2f": xw2f, **in_b_partial[c]})

    nc_b = _build_b(Kg, cs, nblk)
    res_b = run_bass_kernel_spmd(nc_b, in_b, list(range(8)), trace=trace,
                                 tmpdir=tempfile.mkdtemp(prefix="gnn_b_") if trace else None)
    if trace and res_b.exec_time_ns:
        LAST_EXEC_NS.append(res_b.exec_time_ns)

    def full_out(pair):
        return np.concatenate([
            res_b.results[2 * pair]["out_own"][:HALF],
            res_b.results[2 * pair + 1]["out_own"][:HALF]])

    jl, jp, bl, bp = full_out(0), full_out(1), full_out(2), full_out(3)
    res_l = np.concatenate([res_a.results[0]["res_own"][:HALF],
                            res_a.results[1]["res_own"][:HALF]]) + np.asarray(inputs["b_res_lnc"], dtype=f32)
    res_p = np.concatenate([res_a.results[2]["res_own"][:HALF],
                            res_a.results[3]["res_own"][:HALF]]) + np.asarray(inputs["b_res_prot"], dtype=f32)
    comb_l = (jl + bl) * 0.5 + res_l
    comb_p = (jp + bp) * 0.5 + res_p
    return (comb_l, comb_p, jl, jp, bl, bp)

